# revision 13
# baseline (speedup 1.0000x reference)
"""BayesianNN (attention over memory + 2-pass genome gemv) on 8 Trainium2 cores.

Strategy (memory-bound problem; QKV weights = 709 MB of the 1.45 GB input):
  * Column-shard (tensor-parallel) the three QKV projection matrices across the
    8 cores; each core streams its 3 x [7808, 976] f32 shard (pre-transposed on
    host so the contraction dim lands on SBUF partitions) through a SWDGE
    cast-DMA to fp16 (~line-rate) and matmuls against a resident fp16 x^T with
    f32 PSUM accumulation.
  * Biases are folded into the matmul via an extra contraction row
    (x^T row D == 1.0, W^T row D == bias).
  * The [N,N] genome matrices are only ever needed at columns [D:N] (pass 1:
    vals is zero past D; pass 2: only the last 2 outputs matter), so the host
    slices [7816, 130] views - 12 MB instead of 733 MB - row-sharded to match
    each core's v shard.
  * Single collective: pre1 = w . Y with w = colmean(softmax(scores))
    (replicated) and Y = v_shard^T @ W1_shard (shard-summable), so the partial
    scores [128,128] and Y [128,130] ride ONE AllReduce [128,258]; everything
    after it stays on-chip. ctx/pooled are never materialized.
"""

import numpy as np

D = 7686
M = 128
NH = 128
NO = 2
N = D + NH + NO          # 7816
NCORES = 8
JSH = 976                # padded per-core shard width (16 * 61)
IP = 7808                # padded contraction length (61 * 128); row D is the bias row
NIT = IP // 128          # 61 i-tiles
GCH = [128] * 7 + [80]   # genome/v-shard row chunks of the 976-shard
SQRT_D = float(np.sqrt(np.float32(D)))

_COMPILED = None


def _build_program():
    import concourse.bacc as bacc
    import concourse.tile as tile
    import concourse.mybir as mybir
    from concourse import masks
    from functools import partial

    f32, f16 = mybir.dt.float32, mybir.dt.float16
    AF = mybir.ActivationFunctionType

    nc = bacc.Bacc("TRN2", debug=False, num_devices=NCORES)

    wT = {m: nc.dram_tensor(f"{m}T", [IP, JSH], f32, kind="ExternalInput").ap()
          for m in ("k", "q", "v")}
    xT_d = nc.dram_tensor("xT", [IP, M], f32, kind="ExternalInput").ap()
    g_d = {s: nc.dram_tensor(f"g_{s}", [JSH, NH + NO], f32, kind="ExternalInput").ap()
           for s in ("mu", "sig", "eps")}
    h_d = {s: nc.dram_tensor(f"h_{s}", [NH + NO, NO], f32, kind="ExternalInput").ap()
           for s in ("mu", "sig", "eps")}
    b_d = {s: nc.dram_tensor(f"b_{s}", [NH + NO], f32, kind="ExternalInput").ap()
           for s in ("mu", "sig", "eps")}
    out_d = nc.dram_tensor("out", [NO], f32, kind="ExternalOutput").ap()

    with tile.TileContext(nc) as tc:
        with (
            tc.tile_pool(name="const", bufs=1) as constp,
            tc.tile_pool(name="stream", bufs=16) as streamp,
            tc.tile_pool(name="big", bufs=1) as bigp,
            tc.tile_pool(name="small", bufs=2) as smallp,
            tc.tile_pool(name="gen", bufs=1) as genp,
            tc.tile_pool(name="ps_stream", bufs=2, space="PSUM") as ps_stream,
            tc.tile_pool(name="ps_small", bufs=2, space="PSUM") as ps_small,
            tc.tile_pool(name="dram", bufs=1, space="DRAM") as dramp,
        ):
            # ---- resident constants -------------------------------------
            ident = constp.tile([128, 128], f32)
            masks.make_identity(nc, ident[:])
            inv_m = constp.tile([128, 1], f32)
            nc.vector.memset(inv_m[:], 1.0 / M)

            xT_sb = constp.tile([128, NIT * M], f16)
            xT3 = xT_d.rearrange("(t p) m -> p t m", p=128)
            xs3 = xT_sb[:].rearrange("p (t m) -> p t m", m=M)

            def load_xt_tile(c0):
                nc.gpsimd.dma_start(xs3[:, c0:c0 + 1, :], xT3[:, c0:c0 + 1, :])

            # DRAM bounce for the single merged AllReduce: [128, 258] =
            # scores [128,128] ++ Y [128,130]
            ar_in = dramp.tile([M, M + NH + NO], f32)
            ar_out = dramp.tile([M, M + NH + NO], f32)
            groups = [list(range(NCORES))]

            # ---- genome tiles: emitted piecemeal inside the q-stream ----
            gs = []          # sampled W[:D, D:N] row-chunks: [chw, 130] f32
            h2 = []          # sampled W[D:N, N-2:N] split [128]+[2] rows
            b1c = []         # sampled bias[D:N] as columns [128,1] + [2,1]

            def genome_tasks():
                def g_load(ch, chw, box):
                    r0 = ch * 128
                    tl = []
                    for sn in ("mu", "sig", "eps"):
                        t = genp.tile([128, NH + NO], f32, tag=f"g{sn}{ch}",
                                      name=f"g{sn}{ch}")
                        nc.gpsimd.dma_start(t[:chw, :], g_d[sn][r0:r0 + chw, :])
                        tl.append(t)
                    box.append(tl)

                def g_samp(ch, chw, box):
                    gmu, gsg, gep = box.pop()
                    nc.vector.tensor_mul(gsg[:chw, :], gsg[:chw, :], gep[:chw, :])
                    nc.vector.tensor_add(gsg[:chw, :], gsg[:chw, :], gmu[:chw, :])
                    gs.append(gsg)

                def hb_task():
                    for part, (r0, rw) in enumerate(((0, NH), (NH, NO))):
                        hmu = genp.tile([128, NO], f32, tag=f"hmu{part}", name=f"hmu{part}")
                        hsg = genp.tile([128, NO], f32, tag=f"hsg{part}", name=f"hsg{part}")
                        hep = genp.tile([128, NO], f32, tag=f"hep{part}", name=f"hep{part}")
                        for t, sn in ((hmu, "mu"), (hsg, "sig"), (hep, "eps")):
                            nc.gpsimd.dma_start(t[:rw, :], h_d[sn][r0:r0 + rw, :])
                        nc.vector.tensor_mul(hsg[:rw, :], hsg[:rw, :], hep[:rw, :])
                        nc.vector.tensor_add(hsg[:rw, :], hsg[:rw, :], hmu[:rw, :])
                        h2.append(hsg)

                def bias_task():
                    for part, (r0, rw) in enumerate(((0, NH), (NH, NO))):
                        bmu = genp.tile([128, 1], f32, tag=f"bmu{part}", name=f"bmu{part}")
                        bsg = genp.tile([128, 1], f32, tag=f"bsg{part}", name=f"bsg{part}")
                        bep = genp.tile([128, 1], f32, tag=f"bep{part}", name=f"bep{part}")
                        for t, sn in ((bmu, "mu"), (bsg, "sig"), (bep, "eps")):
                            nc.gpsimd.dma_start(t[:rw, :], b_d[sn][r0:r0 + rw])
                        nc.vector.tensor_mul(bsg[:rw, :], bsg[:rw, :], bep[:rw, :])
                        nc.vector.tensor_add(bsg[:rw, :], bsg[:rw, :], bmu[:rw, :])
                        b1c.append(bsg)

                box = []
                for ch, chw in enumerate(GCH):
                    yield partial(g_load, ch, chw, box)
                    yield partial(g_samp, ch, chw, box)
                yield hb_task
                yield bias_task

            # ---- QKV streaming ------------------------------------------
            qkv_sb = {}
            qkvT_sb = {}

            def stream_mat(mat, before_issue=None):
                ps_a = ps_stream.tile([128, 512], f32, tag="ps_a", name=f"psa_{mat}")
                ps_b = ps_stream.tile([128, JSH - 512], f32, tag="ps_b", name=f"psb_{mat}")
                for it in range(NIT):
                    if before_issue is not None:
                        before_issue(it)
                    wt = streamp.tile([128, JSH], f16, tag="wt", name=f"wt_{mat}_{it}")
                    nc.gpsimd.dma_start(wt[:], wT[mat][it * 128:(it + 1) * 128, :])
                    lhsT = xT_sb[:, it * M:(it + 1) * M]
                    nc.tensor.matmul(ps_a[:], lhsT, wt[:, 0:512],
                                     start=(it == 0), stop=(it == NIT - 1))
                    nc.tensor.matmul(ps_b[:], lhsT, wt[:, 512:JSH],
                                     start=(it == 0), stop=(it == NIT - 1))
                sb = bigp.tile([128, JSH], f32, tag=f"{mat}_sb", name=f"{mat}_sb")
                nc.vector.tensor_copy(sb[:, 0:512], ps_a[:])
                nc.vector.tensor_copy(sb[:, 512:JSH], ps_b[:])
                qkv_sb[mat] = sb

            def transpose_mat(mat):
                # [m, j] -> [j, m] 128-tiles (PE transpose via identity)
                sbT = bigp.tile([128, 8 * 128], f32, tag=f"{mat}T_sb", name=f"{mat}T_sb")
                sb = qkv_sb[mat]
                for jt, jw in enumerate(GCH):
                    psT = ps_small.tile([128, 128], f32, tag="psT", name=f"psT_{mat}{jt}")
                    nc.tensor.transpose(
                        psT[:jw, :], sb[:, jt * 128:jt * 128 + jw], ident[:])
                    nc.vector.tensor_copy(
                        sbT[:jw, jt * 128:(jt + 1) * 128], psT[:jw, :])
                qkvT_sb[mat] = sbT

            # PE warm-up: contiguous dummy matmuls while the first tiles land
            ps_warm = ps_small.tile([128, 512], f32, tag="ps_gen", name="ps_warm")
            for r in range(20):
                nc.tensor.matmul(ps_warm[:], xT_sb[:, 0:128], xT_sb[:, 0:512],
                                 start=True, stop=True, skip_group_check=True)

            stream_mat("k", before_issue=load_xt_tile)
            transpose_mat("k")

            gen_tasks = list(genome_tasks())

            def q_hook(it):
                if it % 3 == 0 and gen_tasks:
                    gen_tasks.pop(0)()

            stream_mat("q", before_issue=q_hook)
            while gen_tasks:
                gen_tasks.pop(0)()
            transpose_mat("q")

            # partial scores over the local j-shard -> AR payload cols 0:128
            ps_s = ps_small.tile([128, 128], f32, tag="psT", name="ps_s")
            for jt, jw in enumerate(GCH):
                nc.tensor.matmul(
                    ps_s[:],
                    qkvT_sb["q"][:jw, jt * 128:jt * 128 + 128],
                    qkvT_sb["k"][:jw, jt * 128:jt * 128 + 128],
                    start=(jt == 0), stop=(jt == 7))
            sc_sb = smallp.tile([128, 128], f32)
            nc.vector.tensor_copy(sc_sb[:], ps_s[:])
            nc.sync.dma_start(ar_in[:, 0:M], sc_sb[:])

            stream_mat("v")
            transpose_mat("v")

            # Y = v_shard^T @ gs  (attention-independent, shard-summable)
            ps_y = ps_small.tile([128, NH + NO], f32, tag="ps_gen", name="ps_y")
            for ch, chw in enumerate(GCH):
                nc.tensor.matmul(
                    ps_y[:], qkvT_sb["v"][:chw, ch * 128:ch * 128 + 128],
                    gs[ch][:chw, :],
                    start=(ch == 0), stop=(ch == 7))
            y_sb = smallp.tile([128, NH + NO], f32)
            nc.vector.tensor_copy(y_sb[:], ps_y[:])
            nc.sync.dma_start(ar_in[:, M:M + NH + NO], y_sb[:])

            # ---- the single AllReduce -----------------------------------
            nc.gpsimd.collective_compute(
                "AllReduce", mybir.AluOpType.add, replica_groups=groups,
                ins=[ar_in.opt()], outs=[ar_out.opt()])
            scf = smallp.tile([128, 128], f32)
            nc.sync.dma_start(scf[:], ar_out[:, 0:M])
            yf = smallp.tile([128, NH + NO], f32)
            nc.sync.dma_start(yf[:], ar_out[:, M:M + NH + NO])

            # softmax over free axis of s/sqrt(D)
            mx = smallp.tile([128, 1], f32)
            nc.vector.tensor_reduce(mx[:], scf[:], axis=mybir.AxisListType.X,
                                    op=mybir.AluOpType.max)
            nc.vector.tensor_scalar_sub(scf[:], scf[:], mx[:])
            att = smallp.tile([128, 128], f32)
            nc.scalar.activation(att[:], scf[:], AF.Exp, scale=1.0 / SQRT_D)
            ssum = smallp.tile([128, 1], f32)
            nc.vector.tensor_reduce(ssum[:], att[:], axis=mybir.AxisListType.X,
                                    op=mybir.AluOpType.add)
            rinv = smallp.tile([128, 1], f32)
            nc.vector.reciprocal(rinv[:], ssum[:])
            nc.vector.tensor_scalar_mul(att[:], att[:], rinv[:])

            # w[m'] = (1/M) sum_m attn[m, m']  -> psum [m', 1]
            ps_w = ps_small.tile([128, 1], f32, tag="psT", name="ps_w")
            nc.tensor.matmul(ps_w[:], att[:], inv_m[:])
            w_sb = smallp.tile([128, 1], f32)
            nc.vector.tensor_copy(w_sb[:], ps_w[:])

            # pre1 as columns: [t,1] = Y_full[:, t-chunk]^T @ w
            pre_lo = ps_small.tile([128, 1], f32, tag="psT", name="pre_lo")
            nc.tensor.matmul(pre_lo[:], yf[:, 0:NH], w_sb[:])
            pre_hi = ps_small.tile([NO, 1], f32, tag="ps_gen", name="pre_hi")
            nc.tensor.matmul(pre_hi[:], yf[:, NH:NH + NO], w_sb[:])

            # h = tanh(pre1 + b1)  (columns); fin = tanh(pre1_hi + h-part + b2)
            h_lo = smallp.tile([128, 1], f32)
            nc.vector.tensor_copy(h_lo[:], pre_lo[:])
            nc.vector.tensor_add(h_lo[:], h_lo[:], b1c[0][:, :])
            nc.scalar.activation(h_lo[:], h_lo[:], AF.Tanh)
            h_hi = smallp.tile([NO, 1], f32)
            nc.vector.tensor_copy(h_hi[:], pre_hi[:])
            nc.vector.tensor_add(h_hi[:], h_hi[:], b1c[1][:NO, :])
            nc.scalar.activation(h_hi[:], h_hi[:], AF.Tanh)

            ps_f = ps_small.tile([NO, 1], f32, tag="ps_gen", name="ps_f")
            nc.tensor.matmul(ps_f[:], h2[0][:NH, :], h_lo[:],
                             start=True, stop=False)
            nc.tensor.matmul(ps_f[:], h2[1][:NO, :], h_hi[:],
                             start=False, stop=True)
            fin = smallp.tile([NO, 1], f32)
            nc.vector.tensor_copy(fin[:], ps_f[:])
            nc.vector.tensor_add(fin[:], fin[:], pre_hi[:])
            nc.vector.tensor_add(fin[:], fin[:], b1c[1][:NO, :])
            nc.scalar.activation(fin[:], fin[:], AF.Tanh)
            nc.sync.dma_start(out_d[:], fin[:])

    nc.compile()
    return nc


def _shard_inputs(inputs):
    x = np.ascontiguousarray(inputs["x"], dtype=np.float32)
    xT = np.zeros((IP, M), np.float32)
    xT[:D, :] = x.T
    xT[D, :] = 1.0                      # bias row

    widths = [min(961, D - 961 * c) for c in range(NCORES)]
    offs = [961 * c for c in range(NCORES)]

    in_maps = []
    for c in range(NCORES):
        off, w = offs[c], widths[c]
        im = {"xT": xT}
        for mat, Wn, bn in (("q", "Wq", "bq"), ("k", "Wk", "bk"), ("v", "Wv", "bv")):
            Wt = np.zeros((IP, JSH), np.float32)
            Wt[:D, :w] = inputs[Wn][off:off + w, :].T
            Wt[D, :w] = inputs[bn][off:off + w]
            im[f"{mat}T"] = Wt
        for s, name in (("mu", "W_mu"), ("sig", "W_sigma"), ("eps", "eps_w")):
            g = np.zeros((JSH, NH + NO), np.float32)
            g[:w, :] = inputs[name][off:off + w, D:N]
            im[f"g_{s}"] = g
            im[f"h_{s}"] = np.ascontiguousarray(
                inputs[name][D:N, N - NO:N], dtype=np.float32)
        for s, name in (("mu", "bias_mu"), ("sig", "bias_sigma"), ("eps", "eps_b")):
            im[f"b_{s}"] = np.ascontiguousarray(inputs[name][D:N], dtype=np.float32)
        in_maps.append(im)
    return in_maps


def _run(inputs, trace=False):
    global _COMPILED
    from concourse.bass_utils import run_bass_kernel_spmd

    if _COMPILED is None:
        _COMPILED = _build_program()
    in_maps = _shard_inputs(inputs)
    res = run_bass_kernel_spmd(
        _COMPILED, in_maps, core_ids=list(range(NCORES)), trace=trace)
    out = np.asarray(res.results[0]["out"], dtype=np.float32).reshape(NO)
    return out, res


def kernel(**inputs):
    out, _ = _run(inputs, trace=False)
    return out


# revision 14
# speedup vs baseline: 1.0320x; 1.0320x over previous
"""BayesianNN (attention over memory + 2-pass genome gemv) on 8 Trainium2 cores.

Strategy (memory-bound problem; QKV weights = 709 MB of the 1.45 GB input):
  * Column-shard (tensor-parallel) the three QKV projection matrices across the
    8 cores; each core streams its 3 x [7808, 976] f32 shard (pre-transposed on
    host so the contraction dim lands on SBUF partitions) through a SWDGE
    cast-DMA to fp16 (~line-rate) and matmuls against a resident fp16 x^T with
    f32 PSUM accumulation.
  * Biases are folded into the matmul via an extra contraction row
    (x^T row D == 1.0, W^T row D == bias).
  * The [N,N] genome matrices are only ever needed at columns [D:N] (pass 1:
    vals is zero past D; pass 2: only the last 2 outputs matter), so the host
    slices [7816, 130] views - 12 MB instead of 733 MB - row-sharded to match
    each core's v shard.
  * Single collective: pre1 = w . Y with w = colmean(softmax(scores))
    (replicated) and Y = v_shard^T @ W1_shard (shard-summable), so the partial
    scores [128,128] and Y [128,130] ride ONE AllReduce [128,258]; everything
    after it stays on-chip. ctx/pooled are never materialized.
"""

import numpy as np

D = 7686
M = 128
NH = 128
NO = 2
N = D + NH + NO          # 7816
NCORES = 8
JSH = 976                # padded per-core shard width (16 * 61)
IP = 7808                # padded contraction length (61 * 128); row D is the bias row
NIT = IP // 128          # 61 i-tiles
GCH = [128] * 7 + [80]   # genome/v-shard row chunks of the 976-shard
SQRT_D = float(np.sqrt(np.float32(D)))

_COMPILED = None


def _build_program():
    import concourse.bacc as bacc
    import concourse.tile as tile
    import concourse.mybir as mybir
    from concourse import masks
    from functools import partial

    f32, f16 = mybir.dt.float32, mybir.dt.float16
    AF = mybir.ActivationFunctionType

    nc = bacc.Bacc("TRN2", debug=False, num_devices=NCORES)

    wT = {m: nc.dram_tensor(f"{m}T", [IP, JSH], f32, kind="ExternalInput").ap()
          for m in ("k", "q", "v")}
    xT_d = nc.dram_tensor("xT", [IP, M], f32, kind="ExternalInput").ap()
    g_d = {s: nc.dram_tensor(f"g_{s}", [JSH, NH + NO], f32, kind="ExternalInput").ap()
           for s in ("mu", "sig", "eps")}
    h_d = {s: nc.dram_tensor(f"h_{s}", [NH + NO, NO], f32, kind="ExternalInput").ap()
           for s in ("mu", "sig", "eps")}
    b_d = {s: nc.dram_tensor(f"b_{s}", [NH + NO], f32, kind="ExternalInput").ap()
           for s in ("mu", "sig", "eps")}
    out_d = nc.dram_tensor("out", [NO], f32, kind="ExternalOutput").ap()

    with tile.TileContext(nc) as tc:
        with (
            tc.tile_pool(name="const", bufs=1) as constp,
            tc.tile_pool(name="stream", bufs=24) as streamp,
            tc.tile_pool(name="big", bufs=1) as bigp,
            tc.tile_pool(name="small", bufs=2) as smallp,
            tc.tile_pool(name="gen", bufs=1) as genp,
            tc.tile_pool(name="ps_stream", bufs=2, space="PSUM") as ps_stream,
            tc.tile_pool(name="ps_small", bufs=2, space="PSUM") as ps_small,
            tc.tile_pool(name="dram", bufs=1, space="DRAM") as dramp,
        ):
            # ---- resident constants -------------------------------------
            ident = constp.tile([128, 128], f32)
            masks.make_identity(nc, ident[:])
            inv_m = constp.tile([128, 1], f32)
            nc.vector.memset(inv_m[:], 1.0 / M)

            xT_sb = constp.tile([128, NIT * M], f16)
            xT3 = xT_d.rearrange("(t p) m -> p t m", p=128)
            xs3 = xT_sb[:].rearrange("p (t m) -> p t m", m=M)

            def load_xt_tile(c0):
                nc.gpsimd.dma_start(xs3[:, c0:c0 + 1, :], xT3[:, c0:c0 + 1, :])

            # DRAM bounce buffers for the two AllReduces
            sc_in = dramp.tile([M, M], f32)
            sc_out = dramp.tile([M, M], f32)
            y_in = dramp.tile([M, NH + NO], f32)
            y_out = dramp.tile([M, NH + NO], f32)
            groups = [list(range(NCORES))]

            # ---- genome tiles: emitted piecemeal inside the q-stream ----
            gs = []          # sampled W[:D, D:N] row-chunks: [chw, 130] f32
            h2 = []          # sampled W[D:N, N-2:N] split [128]+[2] rows
            b1c = []         # sampled bias[D:N] as columns [128,1] + [2,1]

            def genome_tasks():
                def g_load(ch, chw, box):
                    r0 = ch * 128
                    tl = []
                    for sn in ("mu", "sig", "eps"):
                        t = genp.tile([128, NH + NO], f32, tag=f"g{sn}{ch}",
                                      name=f"g{sn}{ch}")
                        nc.gpsimd.dma_start(t[:chw, :], g_d[sn][r0:r0 + chw, :])
                        tl.append(t)
                    box.append(tl)

                def g_samp(ch, chw, box):
                    gmu, gsg, gep = box.pop()
                    nc.vector.tensor_mul(gsg[:chw, :], gsg[:chw, :], gep[:chw, :])
                    nc.vector.tensor_add(gsg[:chw, :], gsg[:chw, :], gmu[:chw, :])
                    gs.append(gsg)

                def hb_task():
                    for part, (r0, rw) in enumerate(((0, NH), (NH, NO))):
                        hmu = genp.tile([128, NO], f32, tag=f"hmu{part}", name=f"hmu{part}")
                        hsg = genp.tile([128, NO], f32, tag=f"hsg{part}", name=f"hsg{part}")
                        hep = genp.tile([128, NO], f32, tag=f"hep{part}", name=f"hep{part}")
                        for t, sn in ((hmu, "mu"), (hsg, "sig"), (hep, "eps")):
                            nc.gpsimd.dma_start(t[:rw, :], h_d[sn][r0:r0 + rw, :])
                        nc.vector.tensor_mul(hsg[:rw, :], hsg[:rw, :], hep[:rw, :])
                        nc.vector.tensor_add(hsg[:rw, :], hsg[:rw, :], hmu[:rw, :])
                        h2.append(hsg)

                def bias_task():
                    for part, (r0, rw) in enumerate(((0, NH), (NH, NO))):
                        bmu = genp.tile([128, 1], f32, tag=f"bmu{part}", name=f"bmu{part}")
                        bsg = genp.tile([128, 1], f32, tag=f"bsg{part}", name=f"bsg{part}")
                        bep = genp.tile([128, 1], f32, tag=f"bep{part}", name=f"bep{part}")
                        for t, sn in ((bmu, "mu"), (bsg, "sig"), (bep, "eps")):
                            nc.gpsimd.dma_start(t[:rw, :], b_d[sn][r0:r0 + rw])
                        nc.vector.tensor_mul(bsg[:rw, :], bsg[:rw, :], bep[:rw, :])
                        nc.vector.tensor_add(bsg[:rw, :], bsg[:rw, :], bmu[:rw, :])
                        b1c.append(bsg)

                box = []
                for ch, chw in enumerate(GCH):
                    yield partial(g_load, ch, chw, box)
                    yield partial(g_samp, ch, chw, box)
                yield hb_task
                yield bias_task

            # ---- QKV streaming ------------------------------------------
            qkv_sb = {}
            qkvT_sb = {}

            def stream_mat(mat, before_issue=None, after_issue=None):
                ps_a = ps_stream.tile([128, 512], f32, tag="ps_a", name=f"psa_{mat}")
                ps_b = ps_stream.tile([128, JSH - 512], f32, tag="ps_b", name=f"psb_{mat}")
                for it in range(NIT):
                    if before_issue is not None:
                        before_issue(it)
                    wt = streamp.tile([128, JSH], f16, tag="wt", name=f"wt_{mat}_{it}")
                    dma = nc.gpsimd.dma_start(wt[:], wT[mat][it * 128:(it + 1) * 128, :])
                    if after_issue is not None:
                        after_issue(dma)
                    lhsT = xT_sb[:, it * M:(it + 1) * M]
                    nc.tensor.matmul(ps_a[:], lhsT, wt[:, 0:512],
                                     start=(it == 0), stop=(it == NIT - 1))
                    nc.tensor.matmul(ps_b[:], lhsT, wt[:, 512:JSH],
                                     start=(it == 0), stop=(it == NIT - 1))
                sb = bigp.tile([128, JSH], f32, tag=f"{mat}_sb", name=f"{mat}_sb")
                nc.vector.tensor_copy(sb[:, 0:512], ps_a[:])
                nc.vector.tensor_copy(sb[:, 512:JSH], ps_b[:])
                qkv_sb[mat] = sb

            def transpose_mat(mat):
                # [m, j] -> [j, m] 128-tiles (PE transpose via identity)
                sbT = bigp.tile([128, 8 * 128], f32, tag=f"{mat}T_sb", name=f"{mat}T_sb")
                sb = qkv_sb[mat]
                for jt, jw in enumerate(GCH):
                    psT = ps_small.tile([128, 128], f32, tag="psT", name=f"psT_{mat}{jt}")
                    nc.tensor.transpose(
                        psT[:jw, :], sb[:, jt * 128:jt * 128 + jw], ident[:])
                    nc.vector.tensor_copy(
                        sbT[:jw, jt * 128:(jt + 1) * 128], psT[:jw, :])
                qkvT_sb[mat] = sbT

            # PE warm-up: contiguous dummy matmuls while the first tiles land
            # (rotating two PSUM banks so the writes pipeline back-to-back)
            ps_warm = [ps_small.tile([128, 512], f32, tag="ps_gen", name=f"ps_warm{i}")
                       for i in range(2)]
            for r in range(28):
                nc.tensor.matmul(ps_warm[r % 2][:], xT_sb[:, 0:128], xT_sb[:, 0:512],
                                 start=True, stop=True, skip_group_check=True)

            stream_mat("k", before_issue=load_xt_tile)
            transpose_mat("k")

            gen_tasks = list(genome_tasks())

            def q_hook(it):
                if it % 3 == 0 and gen_tasks:
                    gen_tasks.pop(0)()

            stream_mat("q", before_issue=q_hook)
            while gen_tasks:
                gen_tasks.pop(0)()
            transpose_mat("q")

            # partial scores over the local j-shard -> AR payload cols 0:128
            ps_s = ps_small.tile([128, 128], f32, tag="psT", name="ps_s")
            for jt, jw in enumerate(GCH):
                nc.tensor.matmul(
                    ps_s[:],
                    qkvT_sb["q"][:jw, jt * 128:jt * 128 + 128],
                    qkvT_sb["k"][:jw, jt * 128:jt * 128 + 128],
                    start=(jt == 0), stop=(jt == 7))
            sc_sb = smallp.tile([128, 128], f32)
            nc.vector.tensor_copy(sc_sb[:], ps_s[:])
            nc.sync.dma_start(sc_in[:], sc_sb[:])

            # scores AllReduce rides mid-v-stream; the deep wt buffer absorbs
            # the Pool-sequencer block while ncfw runs it.
            from concourse.bass import _add_dep_helper
            pend = []

            def v_hook(it):
                if it == 30:
                    cc = nc.gpsimd.collective_compute(
                        "AllReduce", mybir.AluOpType.add, replica_groups=groups,
                        ins=[sc_in.opt()], outs=[sc_out.opt()])
                    pend.append(cc)
                elif it == 31 and pend:
                    pass

            def after_issue(dma):
                while pend:
                    _add_dep_helper(dma.ins, pend.pop().ins, sync=True,
                                    reason="pool-order: scores AR before tail v issues")

            stream_mat("v", before_issue=v_hook, after_issue=after_issue)
            transpose_mat("v")

            # Y = v_shard^T @ gs  (attention-independent, shard-summable)
            ps_y = ps_small.tile([128, NH + NO], f32, tag="ps_gen", name="ps_y")
            for ch, chw in enumerate(GCH):
                nc.tensor.matmul(
                    ps_y[:], qkvT_sb["v"][:chw, ch * 128:ch * 128 + 128],
                    gs[ch][:chw, :],
                    start=(ch == 0), stop=(ch == 7))
            y_sb = smallp.tile([128, NH + NO], f32)
            nc.vector.tensor_copy(y_sb[:], ps_y[:])
            nc.sync.dma_start(y_in[:], y_sb[:])

            nc.gpsimd.collective_compute(
                "AllReduce", mybir.AluOpType.add, replica_groups=groups,
                ins=[y_in.opt()], outs=[y_out.opt()])
            scf = smallp.tile([128, 128], f32)
            nc.sync.dma_start(scf[:], sc_out[:])
            yf = smallp.tile([128, NH + NO], f32)
            nc.sync.dma_start(yf[:], y_out[:])

            # softmax over free axis of s/sqrt(D)
            mx = smallp.tile([128, 1], f32)
            nc.vector.tensor_reduce(mx[:], scf[:], axis=mybir.AxisListType.X,
                                    op=mybir.AluOpType.max)
            nc.vector.tensor_scalar_sub(scf[:], scf[:], mx[:])
            att = smallp.tile([128, 128], f32)
            nc.scalar.activation(att[:], scf[:], AF.Exp, scale=1.0 / SQRT_D)
            ssum = smallp.tile([128, 1], f32)
            nc.vector.tensor_reduce(ssum[:], att[:], axis=mybir.AxisListType.X,
                                    op=mybir.AluOpType.add)
            rinv = smallp.tile([128, 1], f32)
            nc.vector.reciprocal(rinv[:], ssum[:])
            nc.vector.tensor_scalar_mul(att[:], att[:], rinv[:])

            # w[m'] = (1/M) sum_m attn[m, m']  -> psum [m', 1]
            ps_w = ps_small.tile([128, 1], f32, tag="psT", name="ps_w")
            nc.tensor.matmul(ps_w[:], att[:], inv_m[:])
            w_sb = smallp.tile([128, 1], f32)
            nc.vector.tensor_copy(w_sb[:], ps_w[:])

            # pre1 as columns: [t,1] = Y_full[:, t-chunk]^T @ w
            pre_lo = ps_small.tile([128, 1], f32, tag="psT", name="pre_lo")
            nc.tensor.matmul(pre_lo[:], yf[:, 0:NH], w_sb[:])
            pre_hi = ps_small.tile([NO, 1], f32, tag="ps_gen", name="pre_hi")
            nc.tensor.matmul(pre_hi[:], yf[:, NH:NH + NO], w_sb[:])

            # h = tanh(pre1 + b1)  (columns); fin = tanh(pre1_hi + h-part + b2)
            h_lo = smallp.tile([128, 1], f32)
            nc.vector.tensor_copy(h_lo[:], pre_lo[:])
            nc.vector.tensor_add(h_lo[:], h_lo[:], b1c[0][:, :])
            nc.scalar.activation(h_lo[:], h_lo[:], AF.Tanh)
            h_hi = smallp.tile([NO, 1], f32)
            nc.vector.tensor_copy(h_hi[:], pre_hi[:])
            nc.vector.tensor_add(h_hi[:], h_hi[:], b1c[1][:NO, :])
            nc.scalar.activation(h_hi[:], h_hi[:], AF.Tanh)

            ps_f = ps_small.tile([NO, 1], f32, tag="ps_gen", name="ps_f")
            nc.tensor.matmul(ps_f[:], h2[0][:NH, :], h_lo[:],
                             start=True, stop=False)
            nc.tensor.matmul(ps_f[:], h2[1][:NO, :], h_hi[:],
                             start=False, stop=True)
            fin = smallp.tile([NO, 1], f32)
            nc.vector.tensor_copy(fin[:], ps_f[:])
            nc.vector.tensor_add(fin[:], fin[:], pre_hi[:])
            nc.vector.tensor_add(fin[:], fin[:], b1c[1][:NO, :])
            nc.scalar.activation(fin[:], fin[:], AF.Tanh)
            nc.sync.dma_start(out_d[:], fin[:])

    nc.compile()
    return nc


def _shard_inputs(inputs):
    x = np.ascontiguousarray(inputs["x"], dtype=np.float32)
    xT = np.zeros((IP, M), np.float32)
    xT[:D, :] = x.T
    xT[D, :] = 1.0                      # bias row

    widths = [min(961, D - 961 * c) for c in range(NCORES)]
    offs = [961 * c for c in range(NCORES)]

    in_maps = []
    for c in range(NCORES):
        off, w = offs[c], widths[c]
        im = {"xT": xT}
        for mat, Wn, bn in (("q", "Wq", "bq"), ("k", "Wk", "bk"), ("v", "Wv", "bv")):
            Wt = np.zeros((IP, JSH), np.float32)
            Wt[:D, :w] = inputs[Wn][off:off + w, :].T
            Wt[D, :w] = inputs[bn][off:off + w]
            im[f"{mat}T"] = Wt
        for s, name in (("mu", "W_mu"), ("sig", "W_sigma"), ("eps", "eps_w")):
            g = np.zeros((JSH, NH + NO), np.float32)
            g[:w, :] = inputs[name][off:off + w, D:N]
            im[f"g_{s}"] = g
            im[f"h_{s}"] = np.ascontiguousarray(
                inputs[name][D:N, N - NO:N], dtype=np.float32)
        for s, name in (("mu", "bias_mu"), ("sig", "bias_sigma"), ("eps", "eps_b")):
            im[f"b_{s}"] = np.ascontiguousarray(inputs[name][D:N], dtype=np.float32)
        in_maps.append(im)
    return in_maps


def _run(inputs, trace=False):
    global _COMPILED
    from concourse.bass_utils import run_bass_kernel_spmd

    if _COMPILED is None:
        _COMPILED = _build_program()
    in_maps = _shard_inputs(inputs)
    res = run_bass_kernel_spmd(
        _COMPILED, in_maps, core_ids=list(range(NCORES)), trace=trace)
    out = np.asarray(res.results[0]["out"], dtype=np.float32).reshape(NO)
    return out, res


def kernel(**inputs):
    out, _ = _run(inputs, trace=False)
    return out
